# revision 19
# baseline (speedup 1.0000x reference)
"""Trainium2 Bass kernel for the adapted C-Mamba block.

Self-contained: takes FULL inputs as numpy arrays, shards the flattened
batch*num_nodes sequence axis across 8 NeuronCores (data parallel), runs a
feature-major Bass/Tile kernel per core, and gathers the full output.

Per-core dataflow (S=128 sequences, T=64 steps, R=S*T=8192 rows):
  - RMSNorm scale per row (norm_w folded into in_proj weights on host)
  - PE transpose to feature-major, written into a zero-padded buffer
    (3 pad cols per sequence) so the causal depthwise conv folds into the
    in_proj matmuls as 4 shifted tap-matmuls accumulating in PSUM
  - x_proj / dt_proj+softplus feature-major
  - SSM scan: partitions = (d-group-of-8, n=16); selector matmuls replicate
    delta/w across n; ScalarE computes dA=exp(A*delta); VectorE multiplies
    and hardware-prefix-scans fused 4-k-tile [128,4096] tiles (B/C
    replicated across k-tiles via stride-0 access patterns)
  - y = sum_n h*C via a block-ones matmul, gating, out_proj, PE transpose
    back to row-major, residual add, store.
The emission is software-pipelined: chunk c+1's norm/projection phases are
emitted between chunk c's two scan halves so ScalarE/PE fill the gaps under
VectorE's scans; the y-sum matmuls trail one d-quad behind the scan.
"""
import os
import sys

import numpy as np

for _p in ("/opt/trn_rl_repo",):
    if _p not in sys.path and os.path.isdir(_p):
        sys.path.append(_p)

import concourse.bacc as bacc  # noqa: E402
import concourse.bass as bass  # noqa: E402
import concourse.tile as tile  # noqa: E402
from concourse import mybir  # noqa: E402
from concourse.bass_utils import run_bass_kernel_spmd  # noqa: E402

F32 = mybir.dt.float32
BF16 = mybir.dt.bfloat16
AF = mybir.ActivationFunctionType
OP = mybir.AluOpType

B, L, N, DM = 8, 64, 128, 128
DS, DCONV, DFF, DTR = 16, 4, 256, 16
EPS = 1e-5
NCORES = 8
SEQ_PER_CORE = (B * N) // NCORES      # 128
R = SEQ_PER_CORE * L                  # 8192 rows per core
T = L                                 # 64
NCH = 8                               # chunks per core
ST = R // NCH                         # 1024 cols per chunk (16 seqs)
RT = ST // 128                        # 8 row-tiles per chunk
NSUB = ST // 512                      # 512-col sub-tiles per chunk
PAD = DCONV - 1                       # 3 zero cols per sequence
SEQW = T + PAD                        # 67
QK = 4                                # k-tiles fused per scan instruction
QW = QK * ST                          # 4096
NQ = 16 // QK                         # quads per half


def build_nc():
    nc = bacc.Bacc()
    x_in = nc.declare_dram_parameter("x", [R, DM], F32, isOutput=False)
    wtap_in = nc.declare_dram_parameter("wtap", [128, 4 * 256], BF16, isOutput=False)
    wz_in = nc.declare_dram_parameter("wz", [128, 256], BF16, isOutput=False)
    bz_in = nc.declare_dram_parameter("bz", [128, 2], F32, isOutput=False)
    cbt_in = nc.declare_dram_parameter("cbt", [128, 2], F32, isOutput=False)
    xpt_in = nc.declare_dram_parameter("xpt", [128, 96], BF16, isOutput=False)
    dtt_in = nc.declare_dram_parameter("dtt", [16, 256], BF16, isOutput=False)
    dtb_in = nc.declare_dram_parameter("dtb", [128, 2], F32, isOutput=False)
    acol_in = nc.declare_dram_parameter("acol", [128, 32], F32, isOutput=False)
    sel16_in = nc.declare_dram_parameter("sel16", [48, 256], BF16, isOutput=False)
    sel8_in = nc.declare_dram_parameter("sel8", [128, 2048], BF16, isOutput=False)
    lsum_in = nc.declare_dram_parameter("lsum", [128, 2048], BF16, isOutput=False)
    identb_in = nc.declare_dram_parameter("identb", [128, 128], BF16, isOutput=False)
    dd_in = nc.declare_dram_parameter("dd", [128, 256], BF16, isOutput=False)
    owt_in = nc.declare_dram_parameter("owt", [128, 256], BF16, isOutput=False)
    opb_in = nc.declare_dram_parameter("opb", [128, 1], F32, isOutput=False)
    misc_in = nc.declare_dram_parameter("misc", [128, 3], F32, isOutput=False)
    out_dram = nc.declare_dram_parameter("out", [R, DM], F32, isOutput=True)

    x_pnd = x_in[:].rearrange("(n p) d -> p n d", p=128)
    out_view = out_dram[:].rearrange("(n p) d -> n p d", p=128)

    with tile.TileContext(nc) as tc:
        with (
            tc.tile_pool(name="const", bufs=1) as cpool,
            tc.tile_pool(name="xrows", bufs=3) as xrows_pool,
            tc.tile_pool(name="xn", bufs=2) as xn_pool,
            tc.tile_pool(name="small", bufs=4) as small,
            tc.tile_pool(name="xt", bufs=2) as xt_pool,
            tc.tile_pool(name="big", bufs=2) as big,
            tc.tile_pool(name="big1", bufs=2) as big1,
            tc.tile_pool(name="dbcp", bufs=2) as dbc_pool,
            tc.tile_pool(name="bc", bufs=2) as bc_pool,
            tc.tile_pool(name="scan", bufs=2) as scan_pool,
            tc.tile_pool(name="epool", bufs=2) as e_pool,
            tc.tile_pool(name="orow", bufs=6) as orow_pool,
            tc.tile_pool(name="mm", bufs=2, space="PSUM") as mm_psum,
            tc.tile_pool(name="rep", bufs=2, space="PSUM") as rep_psum,
            tc.tile_pool(name="ypsum", bufs=1, space="PSUM") as y_psum_pool,
        ):
            def cload(name, dram, shape, dt=F32):
                t = cpool.tile(shape, dt, tag=name)
                nc.sync.dma_start(t[:], dram[:])
                return t

            c_wtap = cload("wtap", wtap_in, [128, 4 * 256], BF16)
            c_wz = cload("wz", wz_in, [128, 256], BF16)
            c_bz = cload("bz", bz_in, [128, 2])
            c_cbt = cload("cbt", cbt_in, [128, 2])
            c_xpt = cload("xpt", xpt_in, [128, 96], BF16)
            c_dtt = cload("dtt", dtt_in, [16, 256], BF16)
            c_dtb = cload("dtb", dtb_in, [128, 2])
            c_acol = cload("acol", acol_in, [128, 32])
            c_sel16 = cload("sel16", sel16_in, [48, 256], BF16)
            c_sel8 = cload("sel8", sel8_in, [128, 2048], BF16)
            c_lsum = cpool.tile([128, 2048], BF16, tag="lsum")
            nc.sync.dma_start(c_lsum[:], lsum_in[:])
            c_identb = cload("identb", identb_in, [128, 128], BF16)
            c_dd = cload("dd", dd_in, [128, 256], BF16)
            c_owt = cload("owt", owt_in, [128, 256], BF16)
            c_opb = cload("opb", opb_in, [128, 1])
            c_misc = cload("misc", misc_in, [128, 3])
            nc.const_aps.aps[(F32, 0.0)] = c_misc[:, 0:1]
            nc.const_aps.aps[(F32, EPS)] = c_misc[:, 1:2]
            nc.const_aps.aps[(F32, 1.0)] = c_misc[:, 2:3]

            st = {}  # per-chunk live tiles

            def phase_a(c):
                """load + RMS norm + transpose into padded buffer"""
                xr = xrows_pool.tile([128, RT, 128], F32, tag="xr")
                nc.sync.dma_start(xr[:], x_pnd[:, c * RT:(c + 1) * RT, :])
                ssq = small.tile([128, RT], F32, tag="ssq")
                for rt in range(RT):
                    scr = small.tile([128, 128], F32, tag="sqscr")
                    nc.scalar.activation(
                        scr[:], xr[:, rt, :], AF.Square,
                        accum_out=ssq[:, rt:rt + 1])
                vtmp = small.tile([128, RT], F32, tag="vtmp")
                nc.scalar.activation(vtmp[:], ssq[:], AF.Ln,
                                     bias=EPS, scale=1.0 / DM)
                rstd = small.tile([128, RT], F32, tag="rstd")
                nc.scalar.activation(rstd[:], vtmp[:], AF.Exp, scale=-0.5)
                xn = xn_pool.tile([128, RT, 128], BF16, tag="xn")
                for rt in range(RT):
                    nc.vector.tensor_scalar_mul(
                        xn[:, rt, :], xr[:, rt, :], rstd[:, rt:rt + 1])
                xT = xt_pool.tile([128, 16, SEQW], BF16, tag="xT")
                nc.gpsimd.memset(xT[:, :, 0:PAD], 0.0)
                for rt in range(RT):
                    pt = mm_psum.tile([128, 128], BF16, tag="mm")
                    nc.tensor.transpose(pt[:], xn[:, rt, :], c_identb[:])
                    nc.scalar.copy(xT[:, 2 * rt:2 * rt + 2, PAD:], pt[:])
                st[c] = {"xr": xr, "xT": xT}

            def mov(c, sub, j):
                s0 = sub * 8
                return st[c]["xT"][:, s0:s0 + 8, PAD - j:PAD - j + T]

            def phase_b(c):
                """in_proj with conv folded (4 shifted tap matmuls) + x_proj"""
                xcT = big1.tile([128, 2, ST], BF16, tag="xcT")
                zsT = big.tile([128, 2, ST], BF16, tag="zsT")
                for f in range(2):
                    for sub in range(NSUB):
                        sl = slice(sub * 512, (sub + 1) * 512)
                        ps = mm_psum.tile([128, 512], F32, tag="mm")
                        for j in range(DCONV):
                            nc.tensor.matmul(
                                ps[:],
                                c_wtap[:, j * 256 + f * 128:
                                       j * 256 + (f + 1) * 128],
                                mov(c, sub, j), start=(j == 0), stop=(j == 3))
                        nc.scalar.activation(
                            xcT[:, f, sl], ps[:], AF.Silu,
                            bias=c_cbt[:, f:f + 1])
                for f in range(2):
                    for sub in range(NSUB):
                        sl = slice(sub * 512, (sub + 1) * 512)
                        ps = mm_psum.tile([128, 512], F32, tag="mm")
                        nc.tensor.matmul(
                            ps[:], c_wz[:, f * 128:(f + 1) * 128],
                            mov(c, sub, 0), start=True, stop=True)
                        nc.scalar.activation(
                            zsT[:, f, sl], ps[:], AF.Silu,
                            bias=c_bz[:, f:f + 1])
                dbc = dbc_pool.tile([48, ST], BF16, tag="dbc")
                for sub in range(NSUB):
                    sl = slice(sub * 512, (sub + 1) * 512)
                    ps48 = mm_psum.tile([48, 512], F32, tag="mm")
                    nc.tensor.matmul(ps48[:], c_xpt[:, 0:48], xcT[:, 0, sl],
                                     start=True, stop=False)
                    nc.tensor.matmul(ps48[:], c_xpt[:, 48:96], xcT[:, 1, sl],
                                     start=False, stop=True)
                    nc.scalar.copy(dbc[:, sl], ps48[:])
                st[c].update(xcT=xcT, zsT=zsT, dbc=dbc)

            def phase_c(c):
                """dt_proj + softplus, w = delta*xc, B/C replication"""
                dbc = st[c]["dbc"]
                dw = big.tile([128, 2, 2, ST], BF16, tag="dw")
                spts = []
                for h in range(2):
                    for sub in range(NSUB):
                        sl = slice(sub * 512, (sub + 1) * 512)
                        ps = mm_psum.tile([128, 512], F32, tag="mm")
                        nc.tensor.matmul(
                            ps[:], c_dtt[:, h * 128:(h + 1) * 128],
                            dbc[0:16, sl], start=True, stop=True)
                        spt = small.tile([128, 512], F32, tag="sptmp")
                        nc.scalar.activation(spt[:], ps[:], AF.Exp,
                                             bias=c_dtb[:, h:h + 1])
                        spts.append((spt, h, sl))
                for spt, h, sl in spts:
                    nc.scalar.activation(dw[:, h, 0, sl], spt[:],
                                         AF.Ln, bias=1.0)
                nc.vector.tensor_mul(dw[:, :, 1, :], dw[:, :, 0, :],
                                     st[c]["xcT"][:])
                reps = []
                for name, si in (("Brep", 0), ("Crep", 1)):
                    sb = bc_pool.tile([128, ST], BF16, tag=name)
                    for sub in range(NSUB):
                        sl = slice(sub * 512, (sub + 1) * 512)
                        psr = mm_psum.tile([128, 512], F32, tag="mm")
                        nc.tensor.matmul(
                            psr[:], c_sel16[:, si * 128:(si + 1) * 128],
                            dbc[:, sl], start=True, stop=True)
                        nc.scalar.copy(sb[:, sl], psr[:])
                    reps.append(sb)
                st[c].update(dw=dw, Brep=reps[0], Crep=reps[1])

            def make_prod(c):
                dw = st[c]["dw"]
                NQT = 2 * NQ
                tiles = [None] * NQT

                def prod(Q):
                    h, q = Q // NQ, Q % NQ
                    dA4 = scan_pool.tile([128, QW], BF16, tag="dA", name="dA4")
                    wrs4 = scan_pool.tile([128, QW], BF16, tag="wrs",
                                          name="wrs4")
                    for i in range(QK):
                        kk = q * QK + i
                        k = h * 16 + kk
                        po = slice(i * ST, (i + 1) * ST)
                        selk = c_sel8[:, kk * 128:(kk + 1) * 128]
                        dpsum = rep_psum.tile([128, ST], F32, tag="rep",
                                              name="dpsum")
                        for sub in range(NSUB):
                            sl = slice(sub * 512, (sub + 1) * 512)
                            nc.tensor.matmul(
                                dpsum[:, sl], selk, dw[:, h, 0, sl],
                                start=True, stop=True)
                        nc.scalar.activation(
                            dA4[:, po], dpsum[:], AF.Exp,
                            scale=c_acol[:, k:k + 1])
                        wpsum = rep_psum.tile([128, ST], F32, tag="rep",
                                              name="wpsum")
                        for sub in range(NSUB):
                            sl = slice(sub * 512, (sub + 1) * 512)
                            nc.tensor.matmul(
                                wpsum[:, sl], selk, dw[:, h, 1, sl],
                                start=True, stop=True)
                        nc.scalar.copy(wrs4[:, po], wpsum[:])
                    tiles[Q] = (dA4, wrs4)
                return prod, tiles

            def scan_chunk(c, prodstate, interleave=None):
                """8-quad pipelined scan: production (PE reps + ACT exp/copy)
                runs one quad ahead of DVE consumption; ysum trails one quad;
                per-half D*xc seeds and y2 gating slotted between."""
                dw, Brep, Crep = st[c]["dw"], st[c]["Brep"], st[c]["Crep"]
                y2 = st[c]["y2"]
                NQT = 2 * NQ
                prod, tiles = prodstate
                msbs = [None] * NQT
                ypss = [None, None]

                def _unused_prod(Q):
                    h, q = Q // NQ, Q % NQ
                    dA4 = scan_pool.tile([128, QW], BF16, tag="dA", name="dA4")
                    wrs4 = scan_pool.tile([128, QW], BF16, tag="wrs",
                                          name="wrs4")
                    for i in range(QK):
                        kk = q * QK + i
                        k = h * 16 + kk
                        po = slice(i * ST, (i + 1) * ST)
                        selk = c_sel8[:, kk * 128:(kk + 1) * 128]
                        dpsum = rep_psum.tile([128, ST], F32, tag="rep",
                                              name="dpsum")
                        for sub in range(NSUB):
                            sl = slice(sub * 512, (sub + 1) * 512)
                            nc.tensor.matmul(
                                dpsum[:, sl], selk, dw[:, h, 0, sl],
                                start=True, stop=True)
                        nc.scalar.activation(
                            dA4[:, po], dpsum[:], AF.Exp,
                            scale=c_acol[:, k:k + 1])
                        wpsum = rep_psum.tile([128, ST], F32, tag="rep",
                                              name="wpsum")
                        for sub in range(NSUB):
                            sl = slice(sub * 512, (sub + 1) * 512)
                            nc.tensor.matmul(
                                wpsum[:, sl], selk, dw[:, h, 1, sl],
                                start=True, stop=True)
                        nc.scalar.copy(wrs4[:, po], wpsum[:])
                    tiles[Q] = (dA4, wrs4)

                def cons(Q):
                    dA4, wrs4 = tiles[Q]
                    dA3 = dA4[:].rearrange("p (s t) -> p s t", t=T)
                    nc.vector.memset(dA3[:, :, 0:1], 0.0)
                    g4 = scan_pool.tile([128, QW], BF16, tag="g", name="g4")
                    nc.vector.tensor_mul(
                        g4[:].rearrange("p (q s) -> p q s", q=QK),
                        wrs4[:].rearrange("p (q s) -> p q s", q=QK),
                        Brep[:].unsqueeze(1).broadcast_to([128, QK, ST]))
                    hsb4 = scan_pool.tile([128, QW], BF16, tag="h", name="h4")
                    nc.vector.tensor_tensor_scan(
                        hsb4[:], dA4[:], g4[:], 0.0,
                        op0=OP.mult, op1=OP.add)
                    msb4 = scan_pool.tile([128, QW], BF16, tag="m", name="m4")
                    nc.vector.tensor_mul(
                        msb4[:].rearrange("p (q s) -> p q s", q=QK),
                        hsb4[:].rearrange("p (q s) -> p q s", q=QK),
                        Crep[:].unsqueeze(1).broadcast_to([128, QK, ST]))
                    msbs[Q] = msb4

                def ddiag(h):
                    yps = y_psum_pool.tile([128, ST], F32, tag="y", name="yps")
                    for sub in range(NSUB):
                        ysl = slice(sub * 512, (sub + 1) * 512)
                        nc.tensor.matmul(yps[:, ysl],
                                         c_dd[:, h * 128:(h + 1) * 128],
                                         st[c]["xcT"][:, h, ysl],
                                         start=True, stop=False)
                    ypss[h] = yps

                def emit_ysum(Q):
                    h, q = Q // NQ, Q % NQ
                    for i in range(QK):
                        kk = q * QK + i
                        for sub in range(NSUB):
                            sl = slice(i * ST + sub * 512,
                                       i * ST + (sub + 1) * 512)
                            ysl = slice(sub * 512, (sub + 1) * 512)
                            nc.tensor.matmul(
                                ypss[h][:, ysl],
                                c_lsum[:, kk * 128:(kk + 1) * 128],
                                msbs[Q][:, sl],
                                start=False, stop=(kk == 15))

                def y2mul(h):
                    nc.vector.tensor_mul(y2[:, h, :], ypss[h][:],
                                         st[c]["zsT"][:, h, :])

                for Q in range(NQT + 1):
                    if 0 < Q < NQT:
                        prod(Q)
                    if Q >= 1 and Q - 1 < NQT:
                        cons(Q - 1)
                    if Q == NQ + 2:
                        y2mul(0)
                    if Q == 2:
                        ddiag(0)
                    if Q == NQ + 2:
                        ddiag(1)
                    if Q >= 2 and Q - 2 < NQT - 1:
                        emit_ysum(Q - 2)
                    if (interleave and Q - 1 < len(interleave) and Q >= 1
                            and interleave[Q - 1] is not None):
                        interleave[Q - 1]()

                def drain():
                    emit_ysum(NQT - 1)
                    y2mul(1)
                return drain

            def out_phase(c):
                """out_proj + transpose back + residual + store"""
                y2, xr = st[c]["y2"], st[c]["xr"]
                oT = xt_pool.tile([128, ST], BF16, tag="oT")
                for sub in range(NSUB):
                    sl = slice(sub * 512, (sub + 1) * 512)
                    ps = mm_psum.tile([128, 512], F32, tag="mm")
                    nc.tensor.matmul(ps[:], c_owt[:, 0:128], y2[:, 0, sl],
                                     start=True, stop=False)
                    nc.tensor.matmul(ps[:], c_owt[:, 128:256], y2[:, 1, sl],
                                     start=False, stop=True)
                    nc.scalar.activation(oT[:, sl], ps[:], AF.Identity,
                                         bias=c_opb[:, 0:1])
                for rt in range(RT):
                    pt = mm_psum.tile([128, 128], BF16, tag="mm")
                    nc.tensor.transpose(pt[:], oT[:, rt * 128:(rt + 1) * 128],
                                        c_identb[:])
                    orow = orow_pool.tile([128, 128], F32, tag="orow")
                    nc.vector.tensor_add(orow[:], pt[:], xr[:, rt, :])
                    nc.sync.dma_start(out_view[c * RT + rt], orow[:])
                del st[c]

            # software-pipelined emission
            phase_a(0)
            phase_b(0)
            phase_c(0)
            prev = None  # (drain closure, chunk id) from previous chunk
            ps_cur = None
            for c in range(NCH):
                st[c]["y2"] = big1.tile([128, 2, ST], BF16, tag="y2",
                                        name="y2")
                if c == 0:
                    ps_cur = make_prod(0)
                    ps_cur[0](0)
                slots = []
                if prev is not None:
                    dr, pc = prev
                    slots += [dr, lambda pc=pc: out_phase(pc)]
                else:
                    slots += [None, None]
                if c + 1 < NCH:
                    def hoist(c=c):
                        st["ps_next"] = make_prod(c + 1)
                        st["ps_next"][0](0)
                    slots += [lambda c=c: phase_a(c + 1),
                              None,
                              lambda c=c: phase_b(c + 1),
                              None,
                              lambda c=c: phase_c(c + 1),
                              hoist]
                dr = scan_chunk(c, ps_cur, interleave=slots)
                prev = (dr, c)
                if c + 1 < NCH:
                    ps_cur = st.pop("ps_next")
            dr, pc = prev
            dr()
            out_phase(pc)
    nc.finalize()
    return nc


def host_prep(inp):
    import ml_dtypes
    f = lambda a: np.ascontiguousarray(np.asarray(a, np.float32))
    fb = lambda a: np.ascontiguousarray(
        np.asarray(a, np.float32).astype(ml_dtypes.bfloat16))
    wd = {}
    w1n = (np.asarray(inp["in_proj_w"], np.float32)
           * np.asarray(inp["norm_w"], np.float32)[None, :])  # (512,128)
    cwf = np.asarray(inp["conv_w"], np.float32)[:, 0, :]       # (256,4)
    b1 = np.asarray(inp["in_proj_b"], np.float32)
    wtap = np.zeros((128, 4, 256), np.float32)
    for j in range(DCONV):
        wtap[:, j, :] = (w1n[:256].T) * cwf[:, j][None, :]
    wd["wtap"] = fb(wtap.reshape(128, 4 * 256))
    wd["wz"] = fb(w1n[256:].T)                                 # (128,256)
    wd["bz"] = f(b1[256:].reshape(2, 128).T)
    cbt = (np.asarray(inp["conv_b"], np.float32)
           + cwf.sum(axis=1) * b1[:256])
    wd["cbt"] = f(cbt.reshape(2, 128).T)
    xpt = np.asarray(inp["x_proj_w"], np.float32).T            # (256,48)
    wd["xpt"] = fb(np.concatenate([xpt[:128], xpt[128:]], axis=1))  # (128,96)
    wd["dtt"] = fb(np.asarray(inp["dt_proj_w"], np.float32).T)  # (16,256)
    wd["dtb"] = f(np.asarray(inp["dt_proj_b"], np.float32).reshape(2, 128).T)
    A = -np.exp(np.asarray(inp["A_log"], np.float32))          # (256,16)
    wd["acol"] = f(A.reshape(32, 8 * 16).T.reshape(128, 32))
    p = np.arange(128)
    sel16 = np.zeros((48, 2, 128), np.float32)
    for si, off in ((0, 16), (1, 32)):
        sel16[:, si, :] = (np.arange(48)[:, None] == off + p[None, :] % 16)
    wd["sel16"] = fb(sel16.reshape(48, 256))
    sel8 = np.zeros((128, 16, 128), np.float32)
    for kk in range(16):
        sel8[:, kk, :] = (np.arange(128)[:, None] == 8 * kk + p[None, :] // 16)
    wd["sel8"] = fb(sel8.reshape(128, 2048))
    lsum = np.zeros((128, 16, 128), np.float32)
    for kk in range(16):
        for j in range(8):
            lsum[:, kk, 8 * kk + j] = (p // 16 == j)
    wd["lsum"] = np.ascontiguousarray(
        lsum.reshape(128, 2048).astype(ml_dtypes.bfloat16))
    wd["identb"] = fb(np.eye(128))
    Dh = np.asarray(inp["D"], np.float32)
    dd = np.zeros((128, 2, 128), np.float32)
    for hh in range(2):
        np.fill_diagonal(dd[:, hh, :], Dh[hh * 128:(hh + 1) * 128])
    wd["dd"] = fb(dd.reshape(128, 256))
    wd["owt"] = fb(np.asarray(inp["out_proj_w"], np.float32).T
                  .reshape(2, 128, 128).transpose(1, 0, 2).reshape(128, 256))
    wd["opb"] = f(np.asarray(inp["out_proj_b"], np.float32).reshape(128, 1))
    misc = np.zeros((128, 3), np.float32)
    misc[:, 1] = EPS
    misc[:, 2] = 1.0
    wd["misc"] = misc
    return wd


_CACHE = {}


def kernel(**inputs) -> np.ndarray:
    if "nc" not in _CACHE:
        _CACHE["nc"] = build_nc()
    nc = _CACHE["nc"]
    wd = host_prep(inputs)
    xf = np.ascontiguousarray(
        np.asarray(inputs["x"], np.float32).reshape(B * L * N, DM))
    in_maps = []
    for c in range(NCORES):
        m = dict(wd)
        m["x"] = np.ascontiguousarray(xf[c * R:(c + 1) * R])
        in_maps.append(m)
    res = run_bass_kernel_spmd(nc, in_maps, list(range(NCORES)))
    outs = [np.asarray(res.results[c]["out"]) for c in range(NCORES)]
    return np.concatenate(outs, 0).reshape(B, L, N, DM).astype(np.float32)


if __name__ == "__main__":
    nc = build_nc()
    print("built ok")


# revision 22
# speedup vs baseline: 1.0027x; 1.0027x over previous
"""Trainium2 Bass kernel for the adapted C-Mamba block.

Self-contained: takes FULL inputs as numpy arrays, shards the flattened
batch*num_nodes sequence axis across 8 NeuronCores (data parallel), runs a
feature-major Bass/Tile kernel per core, and gathers the full output.

Per-core dataflow (S=128 sequences, T=64 steps, R=S*T=8192 rows):
  - RMSNorm scale per row (norm_w folded into in_proj weights on host)
  - PE transpose to feature-major, written into a zero-padded buffer
    (3 pad cols per sequence) so the causal depthwise conv folds into the
    in_proj matmuls as 4 shifted tap-matmuls accumulating in PSUM
  - x_proj / dt_proj+softplus feature-major
  - SSM scan: partitions = (d-group-of-8, n=16); selector matmuls replicate
    delta/w across n; ScalarE computes dA=exp(A*delta); VectorE multiplies
    and hardware-prefix-scans fused 4-k-tile [128,4096] tiles (B/C
    replicated across k-tiles via stride-0 access patterns)
  - y = sum_n h*C via a block-ones matmul, gating, out_proj, PE transpose
    back to row-major, residual add, store.
The emission is software-pipelined: chunk c+1's norm/projection phases are
emitted between chunk c's two scan halves so ScalarE/PE fill the gaps under
VectorE's scans; the y-sum matmuls trail one d-quad behind the scan.
"""
import os
import sys

import numpy as np

for _p in ("/opt/trn_rl_repo",):
    if _p not in sys.path and os.path.isdir(_p):
        sys.path.append(_p)

import concourse.bacc as bacc  # noqa: E402
import concourse.bass as bass  # noqa: E402
import concourse.tile as tile  # noqa: E402
from concourse import mybir  # noqa: E402
from concourse.bass_utils import run_bass_kernel_spmd  # noqa: E402

F32 = mybir.dt.float32
BF16 = mybir.dt.bfloat16
AF = mybir.ActivationFunctionType
OP = mybir.AluOpType

B, L, N, DM = 8, 64, 128, 128
DS, DCONV, DFF, DTR = 16, 4, 256, 16
EPS = 1e-5
NCORES = 8
SEQ_PER_CORE = (B * N) // NCORES      # 128
R = SEQ_PER_CORE * L                  # 8192 rows per core
T = L                                 # 64
NCH = 8                               # chunks per core
ST = R // NCH                         # 1024 cols per chunk (16 seqs)
RT = ST // 128                        # 8 row-tiles per chunk
NSUB = ST // 512                      # 512-col sub-tiles per chunk
PAD = DCONV - 1                       # 3 zero cols per sequence
SEQW = T + PAD                        # 67
QK = 4                                # k-tiles fused per scan instruction
QW = QK * ST                          # 4096
NQ = 16 // QK                         # quads per half


def build_nc():
    nc = bacc.Bacc()
    x_in = nc.declare_dram_parameter("x", [R, DM], F32, isOutput=False)
    wtap_in = nc.declare_dram_parameter("wtap", [128, 4 * 256], BF16, isOutput=False)
    wz_in = nc.declare_dram_parameter("wz", [128, 256], BF16, isOutput=False)
    bz_in = nc.declare_dram_parameter("bz", [128, 2], F32, isOutput=False)
    cbt_in = nc.declare_dram_parameter("cbt", [128, 2], F32, isOutput=False)
    xpt_in = nc.declare_dram_parameter("xpt", [128, 96], BF16, isOutput=False)
    dtt_in = nc.declare_dram_parameter("dtt", [16, 256], BF16, isOutput=False)
    dtb_in = nc.declare_dram_parameter("dtb", [128, 2], F32, isOutput=False)
    acol_in = nc.declare_dram_parameter("acol", [128, 32], F32, isOutput=False)
    sel16_in = nc.declare_dram_parameter("sel16", [48, 256], BF16, isOutput=False)
    sel8_in = nc.declare_dram_parameter("sel8", [128, 2048], BF16, isOutput=False)
    lsum_in = nc.declare_dram_parameter("lsum", [128, 2048], BF16, isOutput=False)
    identb_in = nc.declare_dram_parameter("identb", [128, 128], BF16, isOutput=False)
    dd_in = nc.declare_dram_parameter("dd", [128, 256], BF16, isOutput=False)
    owt_in = nc.declare_dram_parameter("owt", [128, 256], BF16, isOutput=False)
    opb_in = nc.declare_dram_parameter("opb", [128, 1], F32, isOutput=False)
    misc_in = nc.declare_dram_parameter("misc", [128, 3], F32, isOutput=False)
    out_dram = nc.declare_dram_parameter("out", [R, DM], F32, isOutput=True)

    x_pnd = x_in[:].rearrange("(n p) d -> p n d", p=128)
    out_view = out_dram[:].rearrange("(n p) d -> n p d", p=128)

    with tile.TileContext(nc) as tc:
        with (
            tc.tile_pool(name="const", bufs=1) as cpool,
            tc.tile_pool(name="xrows", bufs=3) as xrows_pool,
            tc.tile_pool(name="xn", bufs=2) as xn_pool,
            tc.tile_pool(name="small", bufs=4) as small,
            tc.tile_pool(name="xt", bufs=2) as xt_pool,
            tc.tile_pool(name="big", bufs=2) as big,
            tc.tile_pool(name="big1", bufs=2) as big1,
            tc.tile_pool(name="dbcp", bufs=2) as dbc_pool,
            tc.tile_pool(name="bc", bufs=2) as bc_pool,
            tc.tile_pool(name="scan", bufs=2) as scan_pool,
            tc.tile_pool(name="epool", bufs=2) as e_pool,
            tc.tile_pool(name="orow", bufs=6) as orow_pool,
            tc.tile_pool(name="mm", bufs=2, space="PSUM") as mm_psum,
            tc.tile_pool(name="rep", bufs=2, space="PSUM") as rep_psum,
            tc.tile_pool(name="ypsum", bufs=1, space="PSUM") as y_psum_pool,
        ):
            def cload(name, dram, shape, dt=F32):
                t = cpool.tile(shape, dt, tag=name)
                nc.sync.dma_start(t[:], dram[:])
                return t

            c_wtap = cload("wtap", wtap_in, [128, 4 * 256], BF16)
            c_wz = cload("wz", wz_in, [128, 256], BF16)
            c_bz = cload("bz", bz_in, [128, 2])
            c_cbt = cload("cbt", cbt_in, [128, 2])
            c_xpt = cload("xpt", xpt_in, [128, 96], BF16)
            c_dtt = cload("dtt", dtt_in, [16, 256], BF16)
            c_dtb = cload("dtb", dtb_in, [128, 2])
            c_acol = cload("acol", acol_in, [128, 32])
            c_sel16 = cload("sel16", sel16_in, [48, 256], BF16)
            c_sel8 = cload("sel8", sel8_in, [128, 2048], BF16)
            c_lsum = cpool.tile([128, 2048], BF16, tag="lsum")
            nc.sync.dma_start(c_lsum[:], lsum_in[:])
            c_identb = cload("identb", identb_in, [128, 128], BF16)
            c_dd = cload("dd", dd_in, [128, 256], BF16)
            c_owt = cload("owt", owt_in, [128, 256], BF16)
            c_opb = cload("opb", opb_in, [128, 1])
            c_misc = cload("misc", misc_in, [128, 3])
            nc.const_aps.aps[(F32, 0.0)] = c_misc[:, 0:1]
            nc.const_aps.aps[(F32, EPS)] = c_misc[:, 1:2]
            nc.const_aps.aps[(F32, 1.0)] = c_misc[:, 2:3]

            st = {}  # per-chunk live tiles

            # persistent scan dA buffers: chain-start (t=0) columns are
            # zeroed once and never rewritten (exp writes skip them)
            dAbufs = []
            for bi in range(2):
                db = cpool.tile([128, QW], BF16, tag=f"dAbuf{bi}",
                                name=f"dAbuf{bi}")
                nc.gpsimd.memset(
                    db[:].rearrange("p (s t) -> p s t", t=T)[:, :, 0:1], 0.0)
                dAbufs.append(db)

            def phase_a(c):
                """load + RMS norm + transpose into padded buffer"""
                xr = xrows_pool.tile([128, RT, 128], F32, tag="xr")
                nc.sync.dma_start(xr[:], x_pnd[:, c * RT:(c + 1) * RT, :])
                ssq = small.tile([128, RT], F32, tag="ssq")
                for rt in range(RT):
                    scr = small.tile([128, 128], F32, tag="sqscr")
                    nc.scalar.activation(
                        scr[:], xr[:, rt, :], AF.Square,
                        accum_out=ssq[:, rt:rt + 1])
                vtmp = small.tile([128, RT], F32, tag="vtmp")
                nc.scalar.activation(vtmp[:], ssq[:], AF.Ln,
                                     bias=EPS, scale=1.0 / DM)
                rstd = small.tile([128, RT], F32, tag="rstd")
                nc.scalar.activation(rstd[:], vtmp[:], AF.Exp, scale=-0.5)
                xn = xn_pool.tile([128, RT, 128], BF16, tag="xn")
                for rt in range(RT):
                    nc.vector.tensor_scalar_mul(
                        xn[:, rt, :], xr[:, rt, :], rstd[:, rt:rt + 1])
                xT = xt_pool.tile([128, 16, SEQW], BF16, tag="xT")
                nc.gpsimd.memset(xT[:, :, 0:PAD], 0.0)
                for rt in range(RT):
                    pt = mm_psum.tile([128, 128], BF16, tag="mm")
                    nc.tensor.transpose(pt[:], xn[:, rt, :], c_identb[:])
                    nc.scalar.copy(xT[:, 2 * rt:2 * rt + 2, PAD:], pt[:])
                st[c] = {"xr": xr, "xT": xT}

            def mov(c, sub, j):
                s0 = sub * 8
                return st[c]["xT"][:, s0:s0 + 8, PAD - j:PAD - j + T]

            def phase_b(c):
                """in_proj with conv folded (4 shifted tap matmuls) + x_proj"""
                xcT = big1.tile([128, 2, ST], BF16, tag="xcT")
                zsT = big.tile([128, 2, ST], BF16, tag="zsT")
                for f in range(2):
                    for sub in range(NSUB):
                        sl = slice(sub * 512, (sub + 1) * 512)
                        ps = mm_psum.tile([128, 512], F32, tag="mm")
                        for j in range(DCONV):
                            nc.tensor.matmul(
                                ps[:],
                                c_wtap[:, j * 256 + f * 128:
                                       j * 256 + (f + 1) * 128],
                                mov(c, sub, j), start=(j == 0), stop=(j == 3))
                        nc.scalar.activation(
                            xcT[:, f, sl], ps[:], AF.Silu,
                            bias=c_cbt[:, f:f + 1])
                for f in range(2):
                    for sub in range(NSUB):
                        sl = slice(sub * 512, (sub + 1) * 512)
                        ps = mm_psum.tile([128, 512], F32, tag="mm")
                        nc.tensor.matmul(
                            ps[:], c_wz[:, f * 128:(f + 1) * 128],
                            mov(c, sub, 0), start=True, stop=True)
                        nc.scalar.activation(
                            zsT[:, f, sl], ps[:], AF.Silu,
                            bias=c_bz[:, f:f + 1])
                dbc = dbc_pool.tile([48, ST], BF16, tag="dbc")
                for sub in range(NSUB):
                    sl = slice(sub * 512, (sub + 1) * 512)
                    ps48 = mm_psum.tile([48, 512], F32, tag="mm")
                    nc.tensor.matmul(ps48[:], c_xpt[:, 0:48], xcT[:, 0, sl],
                                     start=True, stop=False)
                    nc.tensor.matmul(ps48[:], c_xpt[:, 48:96], xcT[:, 1, sl],
                                     start=False, stop=True)
                    nc.scalar.copy(dbc[:, sl], ps48[:])
                st[c].update(xcT=xcT, zsT=zsT, dbc=dbc)

            def phase_c(c):
                """dt_proj + softplus, w = delta*xc, B/C replication"""
                dbc = st[c]["dbc"]
                dw = big.tile([128, 2, 2, ST], BF16, tag="dw")
                spts = []
                for h in range(2):
                    for sub in range(NSUB):
                        sl = slice(sub * 512, (sub + 1) * 512)
                        ps = mm_psum.tile([128, 512], F32, tag="mm")
                        nc.tensor.matmul(
                            ps[:], c_dtt[:, h * 128:(h + 1) * 128],
                            dbc[0:16, sl], start=True, stop=True)
                        spt = small.tile([128, 512], F32, tag="sptmp")
                        nc.scalar.activation(spt[:], ps[:], AF.Exp,
                                             bias=c_dtb[:, h:h + 1])
                        spts.append((spt, h, sl))
                for spt, h, sl in spts:
                    nc.scalar.activation(dw[:, h, 0, sl], spt[:],
                                         AF.Ln, bias=1.0)
                nc.vector.tensor_mul(dw[:, :, 1, :], dw[:, :, 0, :],
                                     st[c]["xcT"][:])
                reps = []
                for name, si in (("Brep", 0), ("Crep", 1)):
                    sb = bc_pool.tile([128, ST], BF16, tag=name)
                    for sub in range(NSUB):
                        sl = slice(sub * 512, (sub + 1) * 512)
                        psr = mm_psum.tile([128, 512], F32, tag="mm")
                        nc.tensor.matmul(
                            psr[:], c_sel16[:, si * 128:(si + 1) * 128],
                            dbc[:, sl], start=True, stop=True)
                        nc.scalar.copy(sb[:, sl], psr[:])
                    reps.append(sb)
                st[c].update(dw=dw, Brep=reps[0], Crep=reps[1])

            def scan_chunk(c, interleave=None):
                """8-quad pipelined scan: production (PE reps + ACT exp/copy)
                runs one quad ahead of DVE consumption; ysum trails one quad;
                per-half D*xc seeds and y2 gating slotted between."""
                dw, Brep, Crep = st[c]["dw"], st[c]["Brep"], st[c]["Crep"]
                y2 = st[c]["y2"]
                NQT = 2 * NQ
                tiles = [None] * NQT   # (dA4, wrs4)
                msbs = [None] * NQT
                ypss = [None, None]

                def prod(Q):
                    h, q = Q // NQ, Q % NQ
                    dA4 = dAbufs[Q % 2]
                    wrs4 = scan_pool.tile([128, QW], BF16, tag="wrs",
                                          name="wrs4")
                    for i in range(QK):
                        kk = q * QK + i
                        k = h * 16 + kk
                        po = slice(i * ST, (i + 1) * ST)
                        selk = c_sel8[:, kk * 128:(kk + 1) * 128]
                        dpsum = rep_psum.tile([128, ST], F32, tag="rep",
                                              name="dpsum")
                        for sub in range(NSUB):
                            sl = slice(sub * 512, (sub + 1) * 512)
                            nc.tensor.matmul(
                                dpsum[:, sl], selk, dw[:, h, 0, sl],
                                start=True, stop=True)
                        dAq = dA4[:, po].rearrange("p (s t) -> p s t", t=T)
                        dpq = dpsum[:].rearrange("p (s t) -> p s t", t=T)
                        nc.scalar.activation(
                            dAq[:, :, 1:], dpq[:, :, 1:], AF.Exp,
                            scale=c_acol[:, k:k + 1])
                        wpsum = rep_psum.tile([128, ST], F32, tag="rep",
                                              name="wpsum")
                        for sub in range(NSUB):
                            sl = slice(sub * 512, (sub + 1) * 512)
                            nc.tensor.matmul(
                                wpsum[:, sl], selk, dw[:, h, 1, sl],
                                start=True, stop=True)
                        nc.scalar.copy(wrs4[:, po], wpsum[:])
                    tiles[Q] = (dA4, wrs4)

                def cons(Q):
                    dA4, wrs4 = tiles[Q]
                    g4 = scan_pool.tile([128, QW], BF16, tag="g", name="g4")
                    nc.vector.tensor_mul(
                        g4[:].rearrange("p (q s) -> p q s", q=QK),
                        wrs4[:].rearrange("p (q s) -> p q s", q=QK),
                        Brep[:].unsqueeze(1).broadcast_to([128, QK, ST]))
                    hsb4 = scan_pool.tile([128, QW], BF16, tag="h", name="h4")
                    nc.vector.tensor_tensor_scan(
                        hsb4[:], dA4[:], g4[:], 0.0,
                        op0=OP.mult, op1=OP.add)
                    msb4 = scan_pool.tile([128, QW], BF16, tag="m", name="m4")
                    nc.vector.tensor_mul(
                        msb4[:].rearrange("p (q s) -> p q s", q=QK),
                        hsb4[:].rearrange("p (q s) -> p q s", q=QK),
                        Crep[:].unsqueeze(1).broadcast_to([128, QK, ST]))
                    msbs[Q] = msb4

                def ddiag(h):
                    yps = y_psum_pool.tile([128, ST], F32, tag="y", name="yps")
                    for sub in range(NSUB):
                        ysl = slice(sub * 512, (sub + 1) * 512)
                        nc.tensor.matmul(yps[:, ysl],
                                         c_dd[:, h * 128:(h + 1) * 128],
                                         st[c]["xcT"][:, h, ysl],
                                         start=True, stop=False)
                    ypss[h] = yps

                def emit_ysum(Q):
                    h, q = Q // NQ, Q % NQ
                    for i in range(QK):
                        kk = q * QK + i
                        for sub in range(NSUB):
                            sl = slice(i * ST + sub * 512,
                                       i * ST + (sub + 1) * 512)
                            ysl = slice(sub * 512, (sub + 1) * 512)
                            nc.tensor.matmul(
                                ypss[h][:, ysl],
                                c_lsum[:, kk * 128:(kk + 1) * 128],
                                msbs[Q][:, sl],
                                start=False, stop=(kk == 15))

                def y2mul(h):
                    nc.vector.tensor_mul(y2[:, h, :], ypss[h][:],
                                         st[c]["zsT"][:, h, :])

                for Q in range(NQT + 1):
                    if Q < NQT:
                        prod(Q)
                    if Q >= 1 and Q - 1 < NQT:
                        cons(Q - 1)
                    if Q == NQ + 2:
                        y2mul(0)
                    if Q == 2:
                        ddiag(0)
                    if Q == NQ + 2:
                        ddiag(1)
                    if Q >= 2 and Q - 2 < NQT - 1:
                        emit_ysum(Q - 2)
                    if Q == NQT:
                        emit_ysum(NQT - 1)
                    if (interleave and Q - 1 < len(interleave) and Q >= 1
                            and interleave[Q - 1] is not None):
                        interleave[Q - 1]()

                def drain():
                    y2mul(1)
                return drain

            def out_phase(c):
                """out_proj + transpose back + residual + store"""
                y2, xr = st[c]["y2"], st[c]["xr"]
                oT = xt_pool.tile([128, ST], BF16, tag="oT")
                for sub in range(NSUB):
                    sl = slice(sub * 512, (sub + 1) * 512)
                    ps = mm_psum.tile([128, 512], F32, tag="mm")
                    nc.tensor.matmul(ps[:], c_owt[:, 0:128], y2[:, 0, sl],
                                     start=True, stop=False)
                    nc.tensor.matmul(ps[:], c_owt[:, 128:256], y2[:, 1, sl],
                                     start=False, stop=True)
                    nc.scalar.activation(oT[:, sl], ps[:], AF.Identity,
                                         bias=c_opb[:, 0:1])
                for rt in range(RT):
                    pt = mm_psum.tile([128, 128], BF16, tag="mm")
                    nc.tensor.transpose(pt[:], oT[:, rt * 128:(rt + 1) * 128],
                                        c_identb[:])
                    orow = orow_pool.tile([128, 128], F32, tag="orow")
                    nc.vector.tensor_add(orow[:], pt[:], xr[:, rt, :])
                    nc.sync.dma_start(out_view[c * RT + rt], orow[:])
                del st[c]

            # software-pipelined emission
            phase_a(0)
            phase_b(0)
            phase_c(0)
            prev = None  # (drain closure, chunk id) from previous chunk
            for c in range(NCH):
                st[c]["y2"] = big1.tile([128, 2, ST], BF16, tag="y2",
                                        name="y2")
                slots = []
                if prev is not None:
                    dr, pc = prev
                    slots += [dr, lambda pc=pc: out_phase(pc)]
                else:
                    slots += [None, None]
                if c + 1 < NCH:
                    slots += [lambda c=c: phase_a(c + 1),
                              None,
                              lambda c=c: phase_b(c + 1),
                              None,
                              lambda c=c: phase_c(c + 1)]
                dr = scan_chunk(c, interleave=slots)
                prev = (dr, c)
            dr, pc = prev
            dr()
            out_phase(pc)
    nc.finalize()
    return nc


def host_prep(inp):
    import ml_dtypes
    f = lambda a: np.ascontiguousarray(np.asarray(a, np.float32))
    fb = lambda a: np.ascontiguousarray(
        np.asarray(a, np.float32).astype(ml_dtypes.bfloat16))
    wd = {}
    w1n = (np.asarray(inp["in_proj_w"], np.float32)
           * np.asarray(inp["norm_w"], np.float32)[None, :])  # (512,128)
    cwf = np.asarray(inp["conv_w"], np.float32)[:, 0, :]       # (256,4)
    b1 = np.asarray(inp["in_proj_b"], np.float32)
    wtap = np.zeros((128, 4, 256), np.float32)
    for j in range(DCONV):
        wtap[:, j, :] = (w1n[:256].T) * cwf[:, j][None, :]
    wd["wtap"] = fb(wtap.reshape(128, 4 * 256))
    wd["wz"] = fb(w1n[256:].T)                                 # (128,256)
    wd["bz"] = f(b1[256:].reshape(2, 128).T)
    cbt = (np.asarray(inp["conv_b"], np.float32)
           + cwf.sum(axis=1) * b1[:256])
    wd["cbt"] = f(cbt.reshape(2, 128).T)
    xpt = np.asarray(inp["x_proj_w"], np.float32).T            # (256,48)
    wd["xpt"] = fb(np.concatenate([xpt[:128], xpt[128:]], axis=1))  # (128,96)
    wd["dtt"] = fb(np.asarray(inp["dt_proj_w"], np.float32).T)  # (16,256)
    wd["dtb"] = f(np.asarray(inp["dt_proj_b"], np.float32).reshape(2, 128).T)
    A = -np.exp(np.asarray(inp["A_log"], np.float32))          # (256,16)
    wd["acol"] = f(A.reshape(32, 8 * 16).T.reshape(128, 32))
    p = np.arange(128)
    sel16 = np.zeros((48, 2, 128), np.float32)
    for si, off in ((0, 16), (1, 32)):
        sel16[:, si, :] = (np.arange(48)[:, None] == off + p[None, :] % 16)
    wd["sel16"] = fb(sel16.reshape(48, 256))
    sel8 = np.zeros((128, 16, 128), np.float32)
    for kk in range(16):
        sel8[:, kk, :] = (np.arange(128)[:, None] == 8 * kk + p[None, :] // 16)
    wd["sel8"] = fb(sel8.reshape(128, 2048))
    lsum = np.zeros((128, 16, 128), np.float32)
    for kk in range(16):
        for j in range(8):
            lsum[:, kk, 8 * kk + j] = (p // 16 == j)
    wd["lsum"] = np.ascontiguousarray(
        lsum.reshape(128, 2048).astype(ml_dtypes.bfloat16))
    wd["identb"] = fb(np.eye(128))
    Dh = np.asarray(inp["D"], np.float32)
    dd = np.zeros((128, 2, 128), np.float32)
    for hh in range(2):
        np.fill_diagonal(dd[:, hh, :], Dh[hh * 128:(hh + 1) * 128])
    wd["dd"] = fb(dd.reshape(128, 256))
    wd["owt"] = fb(np.asarray(inp["out_proj_w"], np.float32).T
                  .reshape(2, 128, 128).transpose(1, 0, 2).reshape(128, 256))
    wd["opb"] = f(np.asarray(inp["out_proj_b"], np.float32).reshape(128, 1))
    misc = np.zeros((128, 3), np.float32)
    misc[:, 1] = EPS
    misc[:, 2] = 1.0
    wd["misc"] = misc
    return wd


_CACHE = {}


def kernel(**inputs) -> np.ndarray:
    if "nc" not in _CACHE:
        _CACHE["nc"] = build_nc()
    nc = _CACHE["nc"]
    wd = host_prep(inputs)
    xf = np.ascontiguousarray(
        np.asarray(inputs["x"], np.float32).reshape(B * L * N, DM))
    in_maps = []
    for c in range(NCORES):
        m = dict(wd)
        m["x"] = np.ascontiguousarray(xf[c * R:(c + 1) * R])
        in_maps.append(m)
    res = run_bass_kernel_spmd(nc, in_maps, list(range(NCORES)))
    outs = [np.asarray(res.results[c]["out"]) for c in range(NCORES)]
    return np.concatenate(outs, 0).reshape(B, L, N, DM).astype(np.float32)


if __name__ == "__main__":
    nc = build_nc()
    print("built ok")
